# revision 2
# baseline (speedup 1.0000x reference)
"""Trainium2 Bass kernel for CubicSplineAutoregressiveSubsetTransform2d.

Computes, per element (B,C,H,W), a monotone cubic Hermite spline (nsf
cubic_spline forward) parameterized by 34 per-element params
(16 widths, 16 heights, 2 derivs), applied to two inputs x_lower/x_upper.

Algorithmic trick: the spline is monotone increasing, so instead of
searchsorted + gather we use the telescoping identity

    z(x) = sum_k sg_k*(D0_k + u_k*(bc_k - aN_k*u_k)),
    sg_k = clamp(x - CW_{k-1}, 0, w_k),  u_k = sg_k / w_k

where full bins contribute exactly h_k and the partial bin contributes the
local cubic. No masks, no gathers.

Precision split (validated numerically): the knot-position path
(exp_w -> sum -> 1/sum -> widths -> cumsum -> x - cw) must be fp32 (position
errors are amplified by spline slopes up to ~3000x near narrow bins);
everything else (heights, slopes, derivatives, Hermite coefficients, u,
final sum) is h-scaled and safe in fp16, which doubles DVE throughput
(2x_1p tensor_tensor mode) and quadruples tensor_scalar/copy.

Layout: the fp32 width path is s-major [P, S, K] (matches the contiguous
param DMA and lets the segmented scan run along (s k)); all fp16 tiles are
k-major [P, K, S] so k-shifted slices and per-element broadcasts keep
innermost stride 1 (required for the 2x DVE mode). The two x evaluations
share one [P, K, 2, S] tile so each chain op processes both in a single
instruction. Unary work (exp, tanh, relu, +MIN, downcasts) runs on the
otherwise-idle Scalar engine.

Sharding: pure data-parallel over batch dim across 8 NeuronCores.
"""

import sys

import numpy as np

for _p in ("/opt/trn_rl_repo",):
    if _p not in sys.path:
        sys.path.insert(0, _p)

import concourse.bass as bass
import concourse.bacc as bacc
import concourse.mybir as mybir
from concourse import tile
from concourse.bass_utils import run_bass_kernel_spmd

F32 = mybir.dt.float32
F16 = mybir.dt.float16
AX = mybir.AxisListType
OP = mybir.AluOpType
ACT = mybir.ActivationFunctionType

B, C, H, W, K = 32, 3, 128, 128, 16
N_CORES = 8
MIN_BIN = 1e-3
SCALE = 1.0 - MIN_BIN * K  # 0.984


def build_program(n_elems: int, S: int = 96):
    """Build the SPMD Bass program for one core processing n_elems elements."""
    P = 128
    per_tile = P * S
    assert n_elems % per_tile == 0
    T = n_elems // per_tile

    nc = bacc.Bacc()
    xl_d = nc.dram_tensor("x_lower", [n_elems], F32, kind="ExternalInput")
    xu_d = nc.dram_tensor("x_upper", [n_elems], F32, kind="ExternalInput")
    pp_d = nc.dram_tensor("elementwise_params", [n_elems, 2 * K + 2], F32,
                          kind="ExternalInput")
    zl_d = nc.dram_tensor("z_lower", [n_elems], F32, kind="ExternalOutput")
    zu_d = nc.dram_tensor("z_upper", [n_elems], F32, kind="ExternalOutput")

    pr = pp_d[:].rearrange("(t p s) k -> t p s k", p=P, s=S)
    xlr = xl_d[:].rearrange("(t p s) -> t p s", p=P, s=S)
    xur = xu_d[:].rearrange("(t p s) -> t p s", p=P, s=S)
    zlr = zl_d[:].rearrange("(t p s) -> t p s", p=P, s=S)
    zur = zu_d[:].rearrange("(t p s) -> t p s", p=P, s=S)

    with tile.TileContext(nc) as tc:
        with tc.tile_pool(name="cst", bufs=1) as cst, \
             tc.tile_pool(name="io", bufs=2) as io, \
             tc.tile_pool(name="wk", bufs=1) as wk, \
             tc.tile_pool(name="ac", bufs=2) as ac:
            # segment mask for the in-tile fp32 cumsum scan: 0 at k=0, 1 else
            segm = cst.tile([P, S, K], F32, tag="segm")
            nc.vector.memset(segm[:], 1.0)
            nc.vector.memset(segm[:, :, 0:1], 0.0)
            for t in range(T):
                # --- loads -------------------------------------------------
                # raw params: consumed only by ACT (exp/tanh), so issuing the
                # DMA from ACT keeps deps program-order (single HW sem wait).
                raw = io.tile([P, S, 34], F32, tag="raw")
                nc.scalar.dma_start(out=raw[:], in_=pr[t])
                xlt = io.tile([P, S], F32, tag="xl")
                nc.scalar.dma_start(out=xlt[:], in_=xlr[t])
                xut = io.tile([P, S], F32, tag="xu")
                nc.scalar.dma_start(out=xut[:], in_=xur[t])
                # bounce x through ACT so the loads' only consumer is ACT
                xlc = wk.tile([P, S], F32, tag="xlc")
                nc.scalar.copy(xlc[:], xlt[:])
                xuc = wk.tile([P, S], F32, tag="xuc")
                nc.scalar.copy(xuc[:], xut[:])

                # =========== W path: fp32, s-major =========================
                ew = ac.tile([P, S, K], F32, tag="ew")
                nc.scalar.activation(ew[:], raw[:, :, 0:K], ACT.Exp)
                Sw = wk.tile([P, S], F32, tag="Sw")
                nc.vector.reduce_sum(Sw[:], ew[:], axis=AX.X)
                rSw = wk.tile([P, S], F32, tag="rSw")
                nc.vector.reciprocal_approx_fast(rSw[:], Sw[:])
                nc.vector.tensor_scalar(rSw[:], rSw[:], SCALE, None, OP.mult)
                # wt32 = ew * rSw_b + MIN  (broadcast along innermost k: 1x)
                wt32 = wk.tile([P, S, K], F32, tag="wt32")
                rSw_b = rSw[:].unsqueeze(2).broadcast_to([P, S, K])
                nc.vector.tensor_tensor(wt32[:], ew[:], rSw_b, OP.mult)
                nc.scalar.activation(wt32[:], wt32[:], ACT.Copy, bias=MIN_BIN)
                # inclusive prefix cumsum per element (segmented scan)
                cw = wk.tile([P, S, K], F32, tag="cw")
                nc.vector.tensor_tensor_scan(
                    cw[:].rearrange("p s k -> p (s k)"),
                    segm[:].rearrange("p s k -> p (s k)"),
                    wt32[:].rearrange("p s k -> p (s k)"),
                    0.0, OP.mult, OP.add)
                rw32 = wk.tile([P, S, K], F32, tag="rw32")
                nc.vector.reciprocal_approx_fast(rw32[:], wt32[:])
                # fp16 k-major downcasts of wt/rw (ACT; strided writes)
                wt16 = wk.tile([P, K, S], F16, tag="wt16")
                nc.scalar.copy(wt16[:].rearrange("p k s -> p s k"), wt32[:])
                rw16 = wk.tile([P, K, S], F16, tag="rw16")
                nc.scalar.copy(rw16[:].rearrange("p k s -> p s k"), rw32[:])

                # =========== H path: fp16, k-major =========================
                eh = wk.tile([P, K, S], F16, tag="eh")
                nc.scalar.activation(eh[:].rearrange("p k s -> p s k"),
                                     raw[:, :, K:2 * K], ACT.Exp)
                # tree-sum over k (in a scratch tile; fp16 2x adds)
                hs = wk.tile([P, K // 2, S], F16, tag="hs")
                nc.vector.tensor_tensor(hs[:], eh[:, 0:8, :], eh[:, 8:16, :],
                                        OP.add)
                nc.vector.tensor_tensor(hs[:, 0:4, :], hs[:, 0:4, :],
                                        hs[:, 4:8, :], OP.add)
                nc.vector.tensor_tensor(hs[:, 0:2, :], hs[:, 0:2, :],
                                        hs[:, 2:4, :], OP.add)
                Sh = wk.tile([P, S], F32, tag="Sh")
                nc.vector.tensor_tensor(Sh[:], hs[:, 0, :], hs[:, 1, :], OP.add)
                rSh32 = wk.tile([P, S], F32, tag="rSh32")
                nc.vector.reciprocal_approx_fast(rSh32[:], Sh[:])
                rSh = wk.tile([P, S], F16, tag="rSh")
                nc.vector.tensor_scalar(rSh[:], rSh32[:], SCALE, None, OP.mult)
                ht = wk.tile([P, K, S], F16, tag="ht")
                rSh_b = rSh[:].unsqueeze(1).broadcast_to([P, K, S])
                nc.vector.tensor_tensor(ht[:], eh[:], rSh_b, OP.mult)
                nc.vector.tensor_scalar(ht[:], ht[:], MIN_BIN, None, OP.add)

                # =========== slopes + derivatives (fp16, k-major) ==========
                st_ = wk.tile([P, K, S], F16, tag="st")
                nc.vector.tensor_tensor(st_[:], ht[:], rw16[:], OP.mult)
                sL = st_[:, 0:K - 1, :]
                sR = st_[:, 1:K, :]
                wL = wt16[:, 0:K - 1, :]
                wR = wt16[:, 1:K, :]
                m1 = wk.tile([P, K - 1, S], F16, tag="m1")
                nc.vector.tensor_tensor(m1[:], sL, sR, OP.min)
                t1 = wk.tile([P, K - 1, S], F16, tag="t1")
                nc.vector.tensor_tensor(t1[:], wR, sL, OP.mult)
                t2 = wk.tile([P, K - 1, S], F16, tag="t2")
                nc.vector.tensor_tensor(t2[:], wL, sR, OP.mult)
                nc.vector.tensor_tensor(t1[:], t1[:], t2[:], OP.add)
                # den in fp32 directly (mixed-out TT, 1x), recip, downcast
                den32 = wk.tile([P, K - 1, S], F32, tag="den32")
                nc.vector.tensor_tensor(den32[:], wL, wR, OP.add)
                rdn32 = wk.tile([P, K - 1, S], F32, tag="rdn32")
                nc.vector.reciprocal_approx_fast(rdn32[:], den32[:])
                rdn16 = wk.tile([P, K - 1, S], F16, tag="rdn16")
                nc.scalar.copy(rdn16[:], rdn32[:])
                nc.vector.tensor_tensor(t1[:], t1[:], rdn16[:], OP.mult)
                m1d = wk.tile([P, K - 1, S], F16, tag="m1d")
                nc.scalar.mul(m1d[:], m1[:], 2.0)
                dlt = wk.tile([P, K + 1, S], F16, tag="dlt")
                nc.vector.tensor_tensor(dlt[:, 1:K, :], m1d[:], t1[:], OP.min)
                # boundary derivs: d = (1.5*tanh(u/2)+1.5) * s_edge
                e01 = wk.tile([P, 2, S], F16, tag="e01")
                nc.scalar.activation(e01[:].rearrange("p k s -> p s k"),
                                     raw[:, :, 2 * K:2 * K + 2],
                                     ACT.Tanh, scale=0.5)
                nc.vector.tensor_scalar(e01[:], e01[:], 1.5, 1.5,
                                        OP.mult, OP.add)
                nc.vector.tensor_tensor(dlt[:, 0:1, :], e01[:, 0:1, :],
                                        st_[:, 0:1, :], OP.mult)
                nc.vector.tensor_tensor(dlt[:, K:K + 1, :], e01[:, 1:2, :],
                                        st_[:, K - 1:K, :], OP.mult)

                # =========== Hermite coefficients ==========================
                D0 = dlt[:, 0:K, :]
                ds = wk.tile([P, K, S], F16, tag="ds")
                nc.vector.tensor_tensor(ds[:], D0, dlt[:, 1:K + 1, :], OP.add)
                st2 = wk.tile([P, K, S], F16, tag="st2")
                nc.scalar.mul(st2[:], st_[:], 2.0)
                aN = wk.tile([P, K, S], F16, tag="aN")
                nc.vector.tensor_tensor(aN[:], st2[:], ds[:], OP.subtract)
                sm = wk.tile([P, K, S], F16, tag="sm")
                nc.vector.tensor_tensor(sm[:], st_[:], D0, OP.subtract)
                bc = wk.tile([P, K, S], F16, tag="bc")
                nc.vector.tensor_tensor(bc[:], aN[:], sm[:], OP.add)

                # =========== evaluate both x in one [P,K,2,S] stream =======
                # tt_k = x - CW_{k-1}: fp32 subtract (cancellation), fp16 out
                tt2 = wk.tile([P, K, 2, S], F16, tag="tt2")
                for j, xc in ((0, xlc), (1, xuc)):
                    x_b = xc[:].unsqueeze(2).broadcast_to([P, S, K - 1])
                    nc.vector.tensor_tensor(
                        tt2[:, 1:K, j, :].rearrange("p k s -> p s k"),
                        x_b, cw[:, :, 0:K - 1], OP.subtract)
                    nc.vector.tensor_copy(tt2[:, 0:1, j, :].rearrange(
                        "p k s -> p s k"), xc[:].unsqueeze(2))
                # sg = clamp(tt, 0, w): relu on ACT (in place), min on DVE
                nc.scalar.activation(tt2[:], tt2[:], ACT.Relu)
                wt_b = wt16[:].unsqueeze(2).broadcast_to([P, K, 2, S])
                sg2 = wk.tile([P, K, 2, S], F16, tag="sg2")
                nc.vector.tensor_tensor(sg2[:], tt2[:], wt_b, OP.min)
                u2 = wk.tile([P, K, 2, S], F16, tag="u2")
                rw_b = rw16[:].unsqueeze(2).broadcast_to([P, K, 2, S])
                nc.vector.tensor_tensor(u2[:], sg2[:], rw_b, OP.mult)
                # hv = sg*(D0 + u*(bc - aN*u)), coeffs broadcast over j
                aN_b = aN[:].unsqueeze(2).broadcast_to([P, K, 2, S])
                bc_b = bc[:].unsqueeze(2).broadcast_to([P, K, 2, S])
                D0_b = dlt[:, 0:K, :].unsqueeze(2).broadcast_to([P, K, 2, S])
                hv = wk.tile([P, K, 2, S], F16, tag="hv")
                nc.vector.tensor_tensor(hv[:], aN_b, u2[:], OP.mult)
                nc.vector.tensor_tensor(hv[:], bc_b, hv[:], OP.subtract)
                nc.vector.tensor_tensor(hv[:], hv[:], u2[:], OP.mult)
                nc.vector.tensor_tensor(hv[:], hv[:], D0_b, OP.add)
                nc.vector.tensor_tensor(hv[:], hv[:], sg2[:], OP.mult)
                # in-place tree sum over k (writes trail disjoint reads)
                nc.vector.tensor_tensor(hv[:, 0:8], hv[:, 0:8], hv[:, 8:16],
                                        OP.add)
                nc.vector.tensor_tensor(hv[:, 0:4], hv[:, 0:4], hv[:, 4:8],
                                        OP.add)
                nc.vector.tensor_tensor(hv[:, 0:2], hv[:, 0:2], hv[:, 2:4],
                                        OP.add)
                zt2 = wk.tile([P, 2, S], F32, tag="zt2")
                nc.vector.tensor_tensor(zt2[:], hv[:, 0, :, :], hv[:, 1, :, :],
                                        OP.add)
                nc.vector.tensor_scalar(zt2[:], zt2[:], 1.0, 0.0,
                                        OP.min, OP.max)
                # bounce through ACT so the store DMAs are program-order deps
                zb = wk.tile([P, 2, S], F32, tag="zb")
                nc.scalar.copy(zb[:], zt2[:])
                nc.scalar.dma_start(out=zlr[t], in_=zb[:, 0, :])
                nc.scalar.dma_start(out=zur[t], in_=zb[:, 1, :])
    nc.finalize()
    return nc


_PROGRAM_CACHE = {}


def _get_program(n_elems, S=96):
    key = (n_elems, S)
    if key not in _PROGRAM_CACHE:
        _PROGRAM_CACHE[key] = build_program(n_elems, S)
    return _PROGRAM_CACHE[key]


def kernel(x_lower, x_upper, elementwise_params):
    x_lower = np.ascontiguousarray(x_lower, dtype=np.float32)
    x_upper = np.ascontiguousarray(x_upper, dtype=np.float32)
    elementwise_params = np.ascontiguousarray(elementwise_params,
                                              dtype=np.float32)
    Bb = x_lower.shape[0]
    per = Bb // N_CORES
    n_elems = per * C * H * W

    nc = _get_program(n_elems)
    in_maps = []
    for c in range(N_CORES):
        sl = slice(c * per, (c + 1) * per)
        in_maps.append({
            "x_lower": x_lower[sl].reshape(n_elems),
            "x_upper": x_upper[sl].reshape(n_elems),
            "elementwise_params": elementwise_params[sl].reshape(n_elems, 34),
        })
    res = run_bass_kernel_spmd(nc, in_maps, list(range(N_CORES)))
    zl = np.concatenate([r["z_lower"].reshape(per, C, H, W)
                         for r in res.results], axis=0)
    zu = np.concatenate([r["z_upper"].reshape(per, C, H, W)
                         for r in res.results], axis=0)
    return zl, zu


if __name__ == "__main__":
    rng = np.random.default_rng(0)
    xl = rng.random((B, C, H, W), dtype=np.float32)
    xu = rng.random((B, C, H, W), dtype=np.float32)
    pp = rng.standard_normal((B, C, H, W, 34), dtype=np.float32)
    zl, zu = kernel(x_lower=xl, x_upper=xu, elementwise_params=pp)
    print("ok", zl.shape, zu.shape, zl.min(), zl.max())
